# revision 13
# baseline (speedup 1.0000x reference)
"""CapsuleLayer dynamic-routing kernel for 8 trn2 NeuronCores (v2.1).

Problem: B=128, U=8, C=2048, J=32, S=16, 3 routing iterations.
  u_hat[b,c,j,s] = sum_u W[c,j,s,u] x[b,u,c]          (never materialized)
  iter: c=softmax(b over C); s=sum_c c*u_hat; v=squash(s); b+=mean_b(u_hat.v)

Sharding: input capsules C split 8 ways (256/core, 2 partition-ranges "cr").
Per iteration each core computes s-partials over its C-slice as PE matmuls
(contraction (u,c_loc) against an e-scaled W), one AllReduce combines
s-partials + softmax denominators, then squash / b-update are local.

v2 design notes (vs v1, 196us graded):
  - bf16 data path everywhere; 1/C folded into xs and 1/B into xa on the
    host (exact powers of two). DVE ops arranged so every operand is
    2-byte, SBUF-resident, packed-innermost => 2x/4x DVE modes.
  - b-update: ACT drains A from PSUM in bf16, DVE computes W*A products
    (4x), an in-place bf16 pairwise tree over u, then one small
    [p,j,s]->[p,j] reduce per cr. Replaces 128 affine_mul_reduce calls.
  - squash sqrt via DVE bit-hack rsqrt + Newton, so ACT stays on the exp
    table set the whole kernel (kills 6x1.28us LoadActFuncSet reloads).
  - e materialized as [128,J,S] by ACT exp with broadcast input, so the
    W-scale runs on DVE in fast mode; split per u-half so s-matmuls start
    early. Pool (slow Q7 software engine) does no elementwise work.
All cores end with the full (identical) v, so core 0's output is the answer.
"""

import numpy as np

B, U, C, J, S = 128, 8, 2048, 32, 16
N_CORES = 8
C_LOC = C // N_CORES          # 256
NCR = C_LOC // 128            # 2 partition-ranges per core
JS = J * S                    # 512
N_ITER = 3

_cache = {}


def _build(use_ar=True, reps=1, mmdt="bf16"):
    import concourse.bacc as bacc
    import concourse.mybir as mybir
    import concourse.tile as tile

    f32 = mybir.dt.float32
    bf16 = mybir.dt.bfloat16
    f16 = mybir.dt.float16
    i32 = mybir.dt.int32
    AT = mybir.AluOpType
    ACT = mybir.ActivationFunctionType
    X = mybir.AxisListType

    nc = bacc.Bacc("TRN2", target_bir_lowering=False, debug=False,
                   num_devices=N_CORES)

    BSH = B // N_CORES             # 16: output batch slice per core
    SH = BSH * JS + 2 * J          # last-iter ReduceScatter shard size

    # per-core inputs (host pre-sharded/transposed; xs carries 1/C, xa 1/B)
    xs_d = nc.dram_tensor("xs", [128, U, NCR, B], bf16, kind="ExternalInput")
    xa_d = nc.dram_tensor("xa", [B, U, NCR, 128], bf16, kind="ExternalInput")
    wa_d = nc.dram_tensor("wa", [128, U, NCR, J, S], bf16, kind="ExternalInput")

    # each core emits its batch slice; host concatenates
    v_d = nc.dram_tensor("v", [BSH, JS], f32, kind="ExternalOutput")

    AR_N1 = B * JS                 # iter-1 payload: s partials only
    AR_N = B * JS + 2 * J          # iter 2: s partials + D partials [1,64]

    with tile.TileContext(nc) as tc:
        with (
            tc.tile_pool(name="big", bufs=1) as big,
            tc.tile_pool(name="sm", bufs=2) as sm,
            tc.tile_pool(name="ps_s", bufs=1, space="PSUM") as ps_s,
            tc.tile_pool(name="ps_a", bufs=2, space="PSUM") as ps_a,
            tc.tile_pool(name="ps_t", bufs=1, space="PSUM") as ps_t,
            tc.tile_pool(name="dram", bufs=1, space="DRAM") as dram,
        ):
            # ---- resident tensors ----
            xs = big.tile([128, U, NCR, B], bf16, tag="xs")
            xa = big.tile([B, U, NCR, 128], bf16, tag="xa")
            wa = big.tile([128, U, NCR, J, S], bf16, tag="wa")
            ww = big.tile([128, U, NCR, J, S], bf16, tag="ww")
            asb = big.tile([128, U, NCR, J, S], bf16, tag="asb")
            pp = big.tile([128, U, NCR, J, S], bf16, tag="pp")
            ef = big.tile([128, NCR, J, S], bf16, tag="ef")

            # loads: xs first (gates every s-matmul), wa per-u chunks across
            # two queues in consumption order, xa (A-step input) last
            nc.sync.dma_start(xs[:], xs_d[:])
            for u in range(U):
                eng = nc.scalar if u % 2 == 0 else nc.sync
                eng.dma_start(wa[:, u], wa_d[:, u])
            nc.gpsimd.dma_start(xa[:], xa_d[:])

            b_cr = [sm.tile([128, J], f32, tag=f"b{cr}", name=f"b{cr}")
                    for cr in range(NCR)]
            binc_cr = [sm.tile([128, J], f32, tag=f"binc{cr}", name=f"binc{cr}")
                       for cr in range(NCR)]
            ones = sm.tile([128, 1], bf16, tag="ones")
            onesr = sm.tile([1, 128], bf16, tag="onesr")
            nc.vector.memset(ones[:], 1.0)
            nc.vector.memset(onesr[:], 1.0)

            for rep in range(reps):
             for it in range(N_ITER):
                first = it == 0
                last = it == N_ITER - 1
                # f16 collective payloads throughout (randn-scale data sits
                # well inside f16 range; quantization ~2^-11 relative)
                pdt = f16

                # ---- c-weights: e = exp(b) broadcast to [128,J,S] on ACT;
                # ww = wa * e per (cr, u-half) on DVE (4x mode) so the first
                # s-matmul chunks can start early.
                if not first:
                    for cr in range(NCR):
                        nc.scalar.activation(
                            ef[:, cr],
                            b_cr[cr][:].unsqueeze(-1).broadcast_to([128, J, S]),
                            ACT.Exp)
                        e_bc = (ef[:, cr].unsqueeze(1)
                                .broadcast_to([128, U // 2, J, S]))
                        for h in range(2):
                            us = slice(h * U // 2, (h + 1) * U // 2)
                            nc.vector.tensor_tensor(
                                out=ww[:, us, cr],
                                in0=wa[:, us, cr],
                                in1=e_bc,
                                op=AT.mult,
                            )
                    # D partials over local c (partition sum): [1, NCR*J]
                    # via matmul ones^T e on the s=0 column of ef
                    dpart_ps = ps_t.tile([1, NCR * J], f32, tag="tiny")
                    for cr in range(NCR):
                        nc.tensor.matmul(dpart_ps[:, cr * J:(cr + 1) * J],
                                         ones[:], ef[:, cr, :, 0],
                                         start=True, stop=True)
                    if last:
                        # replicate into every ReduceScatter shard tail
                        dpart8 = sm.tile([1, N_CORES * 2 * J], pdt,
                                         tag="dpart8")
                        nc.scalar.mul(
                            dpart8[:].rearrange("o (m k) -> o m k",
                                                k=2 * J),
                            dpart_ps[:].unsqueeze(1)
                            .broadcast_to([1, N_CORES, 2 * J]),
                            1.0 / 64.0)
                    else:
                        dpart = sm.tile([1, NCR * J], pdt, tag="dpart")
                        nc.scalar.mul(dpart[:], dpart_ps[:], 1.0 / 64.0)

                # ---- s partials: accumulate 16 chunk matmuls into PSUM ----
                # emission order (u-major inside half) matches ww-half /
                # wa-chunk readiness
                s_ps = ps_s.tile([B, JS], f32, tag="sps")
                rhs = wa if first else ww
                k = 0
                for h in range(2):
                    for cr in range(NCR):
                        for u in range(h * U // 2, (h + 1) * U // 2):
                            nc.tensor.matmul(
                                s_ps[:],
                                xs[:, u, cr],
                                rhs[:, u, cr].rearrange("p a b -> p (a b)"),
                                start=(k == 0), stop=(k == U * NCR - 1),
                            )
                            k += 1

                # drain in f16 payload dtype (issued on ACT; the bounce DMA
                # rides the same queue so no cross-engine hop)
                s_un = sm.tile([B, JS], pdt, tag="sun")
                nc.scalar.copy(s_un[:], s_ps[:])

                # ---- collective: AllReduce (iters 1-2) / ReduceScatter
                # with per-core batch shards (last iter) ----
                if last:
                    ar_in = dram.tile([1, N_CORES * SH], pdt, tag="ar_in2")
                    rs_out = dram.tile([1, SH], pdt, tag="rs_out")
                    ar_v = ar_in[:].rearrange("o (m r) -> o m r", r=SH)
                    nc.scalar.dma_start(ar_v[0, :, 0:BSH * JS], s_un[:])
                    nc.scalar.dma_start(ar_v[0, :, BSH * JS:], dpart8[:])
                    if use_ar:
                        nc.gpsimd.collective_compute(
                            "ReduceScatter", AT.add,
                            replica_groups=[list(range(N_CORES))],
                            ins=[ar_in[:].opt()], outs=[rs_out[:].opt()],
                        )
                    else:
                        nc.sync.dma_start(rs_out[:], ar_v[0, 0])
                    s_sum = sm.tile([BSH, JS], pdt, tag="ssum_sl")
                    nc.sync.dma_start(s_sum[:], rs_out[0, 0:BSH * JS])
                    dsum = sm.tile([1, NCR * J], pdt, tag="dsum")
                    nc.scalar.dma_start(dsum[:], rs_out[0, BSH * JS:])
                    NB = BSH
                else:
                    n_ar = AR_N1 if first else AR_N
                    ar_in = dram.tile([1, n_ar], pdt, tag=f"ar_in{it}")
                    ar_out = dram.tile([1, n_ar], pdt, tag=f"ar_out{it}",
                                       addr_space="Shared")
                    nc.scalar.dma_start(ar_in[0, 0:B * JS], s_un[:])
                    if not first:
                        nc.scalar.dma_start(ar_in[0, B * JS:], dpart[:])
                    if use_ar:
                        nc.gpsimd.collective_compute(
                            "AllReduce", AT.add,
                            replica_groups=[list(range(N_CORES))],
                            ins=[ar_in[:].opt()], outs=[ar_out[:].opt()],
                        )
                    else:
                        nc.sync.dma_start(ar_out[:], ar_in[:])
                    s_sum = sm.tile([B, JS], pdt, tag="ssum")
                    nc.sync.dma_start(s_sum[:], ar_out[0, 0:B * JS])
                    if not first:
                        dsum = sm.tile([1, NCR * J], pdt, tag="dsum")
                        nc.scalar.dma_start(dsum[:], ar_out[0, B * JS:])
                    NB = B

                if first:
                    s_t = s_sum
                else:
                    # fold cr halves, reciprocal, broadcast to [NB, J]
                    dfold = sm.tile([1, J], f32, tag="dfold")
                    nc.vector.tensor_add(dfold[:], dsum[:, 0:J], dsum[:, J:2 * J])
                    drec = sm.tile([1, J], f32, tag="drec")
                    nc.vector.reciprocal(drec[:], dfold[:])
                    drec16 = sm.tile([1, J], bf16, tag="drec16")
                    nc.vector.tensor_copy(drec16[:], drec[:])
                    drec_ps = ps_t.tile([128, J], f32, tag="tiny")
                    nc.tensor.matmul(drec_ps[:], onesr[:], drec16[:],
                                     start=True, stop=True)
                    # s = s_sum * (C/D[j]); drec carries a 64x factor from
                    # the f16 D pre-scale; xs carries 1/C -> net C/64 here
                    s_t = sm.tile([NB, JS], f32, tag=f"st{int(last)}")
                    nc.vector.scalar_tensor_tensor(
                        out=s_t[:].rearrange("p (a b) -> p a b", b=S),
                        in0=s_sum[:].rearrange("p (a b) -> p a b", b=S),
                        scalar=C / 64.0,
                        in1=drec_ps[0:NB].unsqueeze(-1)
                        .broadcast_to([NB, J, S]),
                        op0=AT.mult,
                        op1=AT.mult,
                    )

                # ---- squash (norm over J axis!) all on DVE ----
                L = int(last)
                sq = sm.tile([NB, JS], f32, tag=f"sq{L}")
                nc.vector.tensor_mul(sq[:], s_t[:], s_t[:])
                msq = sm.tile([NB, S], f32, tag=f"msq{L}")
                nc.vector.tensor_reduce(
                    msq[:], sq[:].rearrange("p (a b) -> p b a", b=S),
                    axis=X.X, op=AT.add)
                # y ~= rsqrt(msq): bit-hack seed + Newton step(s)
                ysh = sm.tile([NB, S], i32, tag=f"ysh{L}")
                nc.vector.tensor_scalar(out=ysh[:], in0=msq[:].bitcast(i32),
                                        scalar1=1, scalar2=None,
                                        op0=AT.arith_shift_right)
                y = sm.tile([NB, S], f32, tag=f"y{L}")
                nc.vector.tensor_scalar(out=y[:].bitcast(i32), in0=ysh[:],
                                        scalar1=0x5F3759DF, scalar2=-1,
                                        op0=AT.subtract, op1=AT.mult)
                yt = sm.tile([NB, S], f32, tag=f"yt{L}")
                yu = sm.tile([NB, S], f32, tag=f"yu{L}")
                for _ in range(2 if last else 1):
                    nc.vector.tensor_mul(yt[:], y[:], y[:])
                    nc.vector.tensor_mul(yt[:], yt[:], msq[:])
                    nc.vector.tensor_scalar(out=yu[:], in0=yt[:],
                                            scalar1=-0.5, scalar2=1.5,
                                            op0=AT.mult, op1=AT.add)
                    nc.vector.tensor_mul(y[:], y[:], yu[:])
                # fmul = msq*y/(1+msq)  (= sqrt(msq)/(1+msq))
                den = sm.tile([NB, S], f32, tag=f"den{L}")
                nc.vector.tensor_scalar_add(den[:], msq[:], 1.0)
                rec = sm.tile([NB, S], f32, tag=f"rec{L}")
                nc.vector.reciprocal(rec[:], den[:])
                fmul = sm.tile([NB, S], f32, tag=f"fmul{L}")
                nc.vector.tensor_mul(fmul[:], msq[:], y[:])
                nc.vector.tensor_mul(fmul[:], fmul[:], rec[:])
                v_t = sm.tile([NB, JS], f32 if last else bf16,
                              tag=f"vt{L}")
                nc.vector.tensor_tensor(
                    out=v_t[:].rearrange("p (a b) -> p a b", b=S),
                    in0=s_t[:].rearrange("p (a b) -> p a b", b=S),
                    in1=fmul[:].unsqueeze(1).broadcast_to([NB, J, S]),
                    op=AT.mult,
                )

                if last:
                    nc.sync.dma_start(v_d[:], v_t[:])
                    break

                # ---- b update: A = (x/B)^T v per (u,cr); binc = sum_{u,s}
                # W*A. ACT drains A (bf16), DVE multiplies (4x) and reduces
                # via in-place pairwise tree over u + small s-reduce.
                # cr0 first so its tail hides under cr1's A-matmuls.
                for cr in range(NCR):
                    for u2 in range(U // 2):
                        a_ps = ps_a.tile([128, 2, JS], f32, tag="aps")
                        for hh in range(2):
                            nc.tensor.matmul(a_ps[:, hh],
                                             xa[:, 2 * u2 + hh, cr],
                                             v_t[:], start=True, stop=True)
                        asl = asb[:, 2 * u2:2 * u2 + 2, cr].rearrange(
                            "p a b c -> p a (b c)")
                        nc.scalar.copy(asl, a_ps[:])
                        nc.vector.tensor_tensor(
                            out=pp[:, 2 * u2:2 * u2 + 2, cr],
                            in0=wa[:, 2 * u2:2 * u2 + 2, cr],
                            in1=asb[:, 2 * u2:2 * u2 + 2, cr],
                            op=AT.mult,
                        )
                    # pairwise tree over u (in-place, bf16, 4x), then
                    # [p,j,s] -> [p,j] add-reduce (f32 accum)
                    nc.vector.tensor_add(pp[:, 0:4, cr], pp[:, 0:4, cr],
                                         pp[:, 4:8, cr])
                    nc.vector.tensor_add(pp[:, 0:2, cr], pp[:, 0:2, cr],
                                         pp[:, 2:4, cr])
                    nc.vector.tensor_add(pp[:, 0, cr], pp[:, 0, cr],
                                         pp[:, 1, cr])
                    nc.vector.tensor_reduce(
                        binc_cr[cr][:], pp[:, 0, cr], axis=X.X, op=AT.add)
                    if first:
                        nc.vector.tensor_copy(b_cr[cr][:], binc_cr[cr][:])
                    else:
                        nc.vector.tensor_add(b_cr[cr][:], b_cr[cr][:],
                                             binc_cr[cr][:])

    nc.compile()
    return nc


def _shard_inputs(x, W, mmdt="bf16"):
    import ml_dtypes
    cast = lambda a: np.ascontiguousarray(a, dtype=ml_dtypes.bfloat16)
    x = np.ascontiguousarray(x, dtype=np.float32)
    W = np.ascontiguousarray(W, dtype=np.float32)
    in_maps = []
    for m in range(N_CORES):
        xc = x[:, :, m * C_LOC:(m + 1) * C_LOC]          # [B, U, 256]
        xr = xc.reshape(B, U, NCR, 128)                  # c_loc -> (cr, p)
        xs = cast(xr.transpose(3, 1, 2, 0) * (1.0 / C))  # [128,U,NCR,B]
        xa = cast(xr * (1.0 / B))                        # [B,U,NCR,128]
        Wc = W[0, m * C_LOC:(m + 1) * C_LOC]             # [256, J, S, U]
        wr = Wc.reshape(NCR, 128, J, S, U)
        wa = cast(wr.transpose(1, 4, 0, 2, 3))           # [128,U,NCR,J,S]
        in_maps.append({"xs": xs, "xa": xa, "wa": wa})
    return in_maps


MMDT = "bf16"


def run(x, W, trace=False):
    from concourse import bass_utils

    if "nc" not in _cache:
        _cache["nc"] = _build(mmdt=MMDT)
    nc = _cache["nc"]
    in_maps = _shard_inputs(x, W, mmdt=MMDT)
    res = bass_utils.run_bass_kernel_spmd(
        nc, in_maps, core_ids=list(range(N_CORES)), trace=trace)
    v = np.concatenate([res.results[m]["v"] for m in range(N_CORES)], axis=0)
    v = v.reshape(B, J, S, 1).astype(np.float32)
    return v, res


def kernel(x, W):
    v, _ = run(x, W)
    return v
